# revision 1
# baseline (speedup 1.0000x reference)
"""Bass/Tile Trainium2 kernel for nn_CrossAttention (B=4, Nq=Nk=2048, D=1024, H=16).

Sharding: 8 cores; core c handles batch b=c//2, query rows [(c%2)*1024, (c%2+1)*1024).

Ragged-sequence optimization: valid keys (attention_mask==1) are packed on the host,
so the kernel only attends over ~Nk/2 keys; pad rows get a -60 additive bias before
exp (softmax over packed keys == masked softmax over the full set). Only the last
`nbias` key tiles can contain pad, so all earlier exps skip the bias operand and
fuse two 512-wide score tiles per activation op.

Per-core pipeline (all matmul operands bf16, fp32 PSUM accumulation):
  upfront: vh_all = [v @ Wv | ones-block] for all heads (N=512 matmuls)
  per head-pair hp (projections double-buffered to overlap with attention):
    kh_T pair = Wk-chunk @ k.T ; qh_T pair = Wq-chunk @ q.T
    scores_T[k, q] per 128-k tile; E = exp(scores*SCALE (+ maskbias[k] on tail))
    av[128, q] = vh_aug.T @ E  -> rows 0-63 value, rows 64-127 softmax denominator
    avT = av[0:64] * recip_approx(av[64:128])
  out = avT_all @ Wp.T + bp
"""
import numpy as np
import ml_dtypes

import concourse.bass as bass
import concourse.mybir as mybir
import concourse.tile as tile
from concourse import bacc
from concourse.bass_utils import run_bass_kernel_spmd

F32 = mybir.dt.float32
BF16 = mybir.dt.bfloat16
NPBF16 = ml_dtypes.bfloat16

B, NQ_FULL, NK_FULL, D, H, DH = 4, 2048, 2048, 1024, 16, 64
SCALE = DH ** -0.5
MASK_NEG = -60.0  # additive bias (post-scale) for pad keys; exp(-60) ~ 9e-27
N_CORES = 8


def _chunks(n, w=512):
    out, j = [], 0
    while j < n:
        out.append((j, min(w, n - j)))
        j += min(w, n - j)
    return out


def build_nc(nq, nk, nbias=2, d=D, h=H):
    """Per-core Bass program. nq = q rows/core, nk = packed key rows,
    nbias = # tail key-tiles that may contain pad rows (get the bias operand)."""
    dh = d // h
    assert dh == 64 and d % 128 == 0 and nk % 128 == 0
    IC = d // 128          # contraction chunks for projections
    OC = d // 128          # output chunks (128 rows each)
    NKT = nk // 128        # key tiles
    QC = max(1, nq // 512)  # q chunks of <=512
    QW = min(nq, 512)      # q chunk width
    HP = h // 2            # head pairs
    nbias = min(nbias, NKT)

    nc = bacc.Bacc("TRN2", target_bir_lowering=False, debug=False)

    # activations arrive pre-transposed from the host: [d, n] layout
    xq = nc.declare_dram_parameter("xq", [d, nq], BF16, isOutput=False)
    xk = nc.declare_dram_parameter("xk", [d, nk], BF16, isOutput=False)
    xv = nc.declare_dram_parameter("xv", [d, nk], BF16, isOutput=False)
    wq = nc.declare_dram_parameter("wq", [d, d], BF16, isOutput=False)  # Wq.T [in, out]
    wk = nc.declare_dram_parameter("wk", [d, d], BF16, isOutput=False)
    wv = nc.declare_dram_parameter("wv", [d, d], BF16, isOutput=False)
    wp = nc.declare_dram_parameter("wp", [d, d], BF16, isOutput=False)
    maskb = nc.declare_dram_parameter("maskb", [128, NKT], F32, isOutput=False)
    bpb = nc.declare_dram_parameter("bpb", [1, d], F32, isOutput=False)
    out = nc.declare_dram_parameter("out", [nq, d], F32, isOutput=True)

    with tile.TileContext(nc) as tc:
        with (
            tc.tile_pool(name="wpool", bufs=2) as wpool,
            tc.tile_pool(name="const", bufs=1) as cpool,
            tc.tile_pool(name="acts", bufs=1) as apool,
            tc.tile_pool(name="xT", bufs=1) as xpool,
            tc.tile_pool(name="mm_ps", bufs=2, space="PSUM") as mmps,
            tc.tile_pool(name="qkpair", bufs=2) as qkpool,
        ):
            # --- constants ---
            maskb_s = cpool.tile([128, NKT], F32, tag="maskb")
            nc.sync.dma_start(out=maskb_s[:, :], in_=maskb[:, :])
            bp_row = cpool.tile([1, d], F32, tag="bp_row")
            nc.sync.dma_start(out=bp_row[:, :], in_=bpb[:, :])
            bp_s = cpool.tile([128, d], F32, tag="bp")
            nc.gpsimd.partition_broadcast(bp_s[:, :], bp_row[:, :])

            avT_s = apool.tile([128, OC, nq], BF16, tag="avT")   # normalized att out, T
            vh_s = apool.tile([128, NKT, h, 128], BF16, tag="vh")  # [vh | ones] per head

            def load_w(wdram, name):
                w_s = wpool.tile([128, IC, d], BF16, tag="W", name=name)
                for ic in range(IC):
                    nc.sync.dma_start(
                        out=w_s[:, ic, :], in_=wdram[ic * 128:(ic + 1) * 128, :]
                    )
                return w_s

            def load_xT(xdram, n, name):
                xT = xpool.tile([128, IC, n], BF16, tag=name, name=name)
                for ic in range(IC):
                    nc.sync.dma_start(
                        out=xT[:, ic, :], in_=xdram[ic * 128:(ic + 1) * 128, :]
                    )
                return xT

            # ---- V projection upfront (all heads, N=512) ----
            wk_s = load_w(wk, "wk_s")
            wv_s = load_w(wv, "wv_s")
            xvT = load_xT(xv, nk, "xvT")
            xqT = load_xT(xq, nq, "xqT")
            xkT = load_xT(xk, nk, "xkT")
            nc.gpsimd.memset(vh_s[:, :, :, dh:], 1.0)
            for kt in range(NKT):
                for half in range(2):
                    ps = mmps.tile([128, 512], F32, tag="ps", name=f"vps{kt}_{half}")
                    for ic in range(IC):
                        nc.tensor.matmul(
                            ps[:, :],
                            xvT[:, ic, kt * 128:(kt + 1) * 128],
                            wv_s[:, ic, half * 512:(half + 1) * 512],
                            start=(ic == 0), stop=(ic == IC - 1),
                        )
                    nc.vector.tensor_copy(
                        vh_s[:, kt, 8 * half:8 * half + 8, 0:dh],
                        ps[:, :].rearrange("p (a b) -> p a b", b=dh),
                    )

            wq_s = load_w(wq, "wq_s")  # takes wv's slot once V-projection drains

            # ---- head-pair loop: K/Q projection + attention ----
            with (
                tc.tile_pool(name="epool", bufs=3) as epool,
                tc.tile_pool(name="sc_ps", bufs=2, space="PSUM") as scps,
                tc.tile_pool(name="av_ps", bufs=2, space="PSUM") as avps,
                tc.tile_pool(name="rpool", bufs=3) as rpool,
                tc.tile_pool(name="avn", bufs=2) as avnpool,
            ):
                for hp in range(HP):
                    # K/Q projections for this pair -> [128, n] (2 heads stacked)
                    kh_p = qkpool.tile([128, nk], BF16, tag="kh", name=f"kh{hp}")
                    for j0, jw in _chunks(nk):
                        ps = mmps.tile([128, 512], F32, tag="ps", name=f"kps{hp}_{j0}")
                        for ic in range(IC):
                            nc.tensor.matmul(
                                ps[:, :jw],
                                wk_s[:, ic, hp * 128:(hp + 1) * 128],
                                xkT[:, ic, j0:j0 + jw],
                                start=(ic == 0), stop=(ic == IC - 1),
                            )
                        nc.vector.tensor_copy(kh_p[:, j0:j0 + jw], ps[:, :jw])
                    qh_p = qkpool.tile([128, nq], BF16, tag="qh", name=f"qh{hp}")
                    for j0, jw in _chunks(nq):
                        ps = mmps.tile([128, 512], F32, tag="ps", name=f"qps{hp}_{j0}")
                        for ic in range(IC):
                            nc.tensor.matmul(
                                ps[:, :jw],
                                wq_s[:, ic, hp * 128:(hp + 1) * 128],
                                xqT[:, ic, j0:j0 + jw],
                                start=(ic == 0), stop=(ic == IC - 1),
                            )
                        nc.vector.tensor_copy(qh_p[:, j0:j0 + jw], ps[:, :jw])

                    # attention for both heads of the pair
                    for j in range(QC):
                        q0 = j * QW
                        es = [
                            epool.tile([128, NKT, QW], BF16, tag="e", name=f"e{hp}_{j}_{hf}")
                            for hf in range(2)
                        ]
                        for kp in range((NKT + 1) // 2):
                            kts = [kt for kt in (2 * kp, 2 * kp + 1) if kt < NKT]
                            pss = [
                                scps.tile([128, 2, 512], F32, tag="sc",
                                          name=f"sc{hp}_{j}_{kp}_{hf}")
                                for hf in range(2)
                            ]
                            # alternate row-groups so the K=64 pairs run concurrently
                            for si, kt in enumerate(kts):
                                for half, ps in enumerate(pss):
                                    p0 = half * 64
                                    nc.tensor.matmul(
                                        ps[:, si, :QW],
                                        kh_p[p0:p0 + 64, kt * 128:(kt + 1) * 128],
                                        qh_p[p0:p0 + 64, q0:q0 + QW],
                                        start=True, stop=True,
                                        tile_position=(p0, 0),
                                    )
                            for half, e in enumerate(es):
                                ps = pss[half]
                                if kts[-1] < NKT - nbias and len(kts) == 2:
                                    nc.scalar.activation(
                                        e[:, kts[0]:kts[0] + 2, :], ps[:, :, :QW],
                                        mybir.ActivationFunctionType.Exp,
                                        bias=0.0, scale=SCALE,
                                    )
                                else:
                                    for si, kt in enumerate(kts):
                                        if kt >= NKT - nbias:
                                            nc.scalar.activation(
                                                e[:, kt, :], ps[:, si, :QW],
                                                mybir.ActivationFunctionType.Exp,
                                                bias=maskb_s[:, kt:kt + 1], scale=SCALE,
                                            )
                                        else:
                                            nc.scalar.activation(
                                                e[:, kt, :], ps[:, si, :QW],
                                                mybir.ActivationFunctionType.Exp,
                                                bias=0.0, scale=SCALE,
                                            )
                        for half, e in enumerate(es):
                            hh = 2 * hp + half
                            av = avps.tile([128, 512], F32, tag="av",
                                           name=f"av{hp}_{j}_{half}")
                            for kt in range(NKT):
                                nc.tensor.matmul(
                                    av[:, :QW],
                                    vh_s[:, kt, hh, :],
                                    e[:, kt, :],
                                    start=(kt == 0), stop=(kt == NKT - 1),
                                )
                            d64 = rpool.tile([128, QW], F32, tag="d64",
                                             name=f"d64_{hp}_{j}_{half}")
                            nc.vector.tensor_copy(d64[64:128, :], av[64:128, :QW])
                            d0 = rpool.tile([64, QW], F32, tag="d0",
                                            name=f"d0_{hp}_{j}_{half}")
                            nc.sync.dma_start(out=d0[:, :], in_=d64[64:128, :])
                            rb0 = rpool.tile([64, QW], F32, tag="rb0",
                                             name=f"rb0_{hp}_{j}_{half}")
                            nc.vector.reciprocal_approx_fast(out=rb0[:, :], in_=d0[:, :])
                            if half == 0:
                                nc.vector.tensor_mul(
                                    avT_s[0:dh, hp, q0:q0 + QW], av[0:dh, :QW], rb0[:, :]
                                )
                            else:
                                avn = avnpool.tile([dh, QW], BF16, tag="avn",
                                                   name=f"avn{hp}_{j}")
                                nc.vector.tensor_mul(avn[:, :], av[0:dh, :QW], rb0[:, :])
                                nc.sync.dma_start(
                                    out=avT_s[64:128, hp, q0:q0 + QW], in_=avn[:, :]
                                )

            # ---- output projection ----
            wp_s = load_w(wp, "wp_s")
            with (
                tc.tile_pool(name="o_ps", bufs=3, space="PSUM") as ops,
                tc.tile_pool(name="obuf", bufs=3) as obuf,
            ):
                for qt in range(nq // 128):
                    for o0, ow in _chunks(d):
                        ps = ops.tile([128, 512], F32, tag="o", name=f"o{qt}_{o0}")
                        for dc in range(OC):
                            nc.tensor.matmul(
                                ps[:, :ow],
                                avT_s[:, dc, qt * 128:(qt + 1) * 128],
                                wp_s[:, dc, o0:o0 + ow],
                                start=(dc == 0), stop=(dc == OC - 1),
                            )
                        ot = obuf.tile([128, 512], F32, tag="ot", name=f"ot{qt}_{o0}")
                        nc.vector.tensor_add(ot[:, :ow], ps[:, :ow], bp_s[:, o0:o0 + ow])
                        nc.sync.dma_start(
                            out=out[qt * 128:(qt + 1) * 128, o0:o0 + ow], in_=ot[:, :ow]
                        )

    nc.compile()
    return nc


def host_prep(q, k, v, attention_mask, Wq, Wk, Wv, Wp, bp, nq_per_core=None):
    """Pack valid keys, slice + cast full inputs into per-core input maps."""
    nq = nq_per_core or (NQ_FULL * B // N_CORES)
    bsz, nk_full = attention_mask.shape
    cores_per_b = N_CORES // bsz
    idxs = [np.flatnonzero(attention_mask[b]) for b in range(bsz)]
    nv_min = min(len(ix) for ix in idxs)
    nk = max(128, -(-max(len(ix) for ix in idxs) // 128) * 128)  # padded packed len
    nkt = nk // 128
    nbias = max(1, -(-(nk - nv_min) // 128))

    wqT = np.ascontiguousarray(Wq.T).astype(NPBF16)
    wkT = np.ascontiguousarray(Wk.T).astype(NPBF16)
    wvT = np.ascontiguousarray(Wv.T).astype(NPBF16)
    wpT = np.ascontiguousarray(Wp.T).astype(NPBF16)
    bpb = np.ascontiguousarray(bp[None, :]).astype(np.float32)

    packed = []
    for b in range(bsz):
        ix = idxs[b]
        kp = np.zeros((nk, k.shape[2]), NPBF16)
        vp = np.zeros((nk, v.shape[2]), NPBF16)
        kp[:len(ix)] = k[b][ix].astype(NPBF16)
        vp[:len(ix)] = v[b][ix].astype(NPBF16)
        mb = np.full(nk, MASK_NEG, np.float32)
        mb[:len(ix)] = 0.0
        packed.append((np.ascontiguousarray(kp.T), np.ascontiguousarray(vp.T),
                       np.ascontiguousarray(mb.reshape(nkt, 128).T)))

    in_maps = []
    for c in range(N_CORES):
        b, qi = divmod(c, cores_per_b)
        kp, vp, mb = packed[b]
        in_maps.append({
            "xq": np.ascontiguousarray(q[b, qi * nq:(qi + 1) * nq].astype(NPBF16).T),
            "xk": kp, "xv": vp,
            "wq": wqT, "wk": wkT, "wv": wvT, "wp": wpT,
            "maskb": mb, "bpb": bpb,
        })
    return in_maps, nk, nbias


_NC_CACHE = {}


def get_nc(nq, nk, nbias=2):
    key = (nq, nk, nbias)
    if key not in _NC_CACHE:
        _NC_CACHE[key] = build_nc(nq, nk, nbias)
    return _NC_CACHE[key]


def kernel(q, k, v, attention_mask, Wq, Wk, Wv, Wp, bp):
    nq = NQ_FULL * B // N_CORES
    in_maps, nk, nbias = host_prep(q, k, v, attention_mask, Wq, Wk, Wv, Wp, bp)
    nc = get_nc(nq, nk, nbias)
    res = run_bass_kernel_spmd(nc, in_maps, core_ids=list(range(N_CORES)))
    cores_per_b = N_CORES // B
    out = np.empty((B, NQ_FULL, D), np.float32)
    for c in range(N_CORES):
        b, qi = divmod(c, cores_per_b)
        out[b, qi * nq:(qi + 1) * nq] = res.results[c]["out"]
    return out



# revision 7
# speedup vs baseline: 1.0386x; 1.0386x over previous
"""Bass/Tile Trainium2 kernel for nn_CrossAttention (B=4, Nq=Nk=2048, D=1024, H=16).

Sharding v2: 8 cores; core c handles batch b=c//2 and head-half (c%2)*8..+8,
over ALL 2048 query rows. K/V/Q projections are computed once per (batch, head)
with no duplication. The output projection contracts only this core's 512 head
dims, producing a partial [2048, 1024] fp16 output; the host adds the two
partials per batch plus the bias.

Ragged keys: valid keys packed on host (~Nk/2), tail tiles get a -60 additive
bias before exp.

Per-core pipeline (bf16 matmul operands, fp32 PSUM):
  scores per (head-pair, q-chunk): K=64 matmuls packed two heads per PE pass
    via tile_position row groups (concurrent pairs, ~2x)
  exp: leading KT_DVE key tiles via a custom DVE op (1+x/64)^64, remaining
    tiles (incl. biased tail) on the scalar engine -> both engines in parallel
  AV accumulates 9 key tiles; denominator via ones-block in vh
  software-pipelined emission: scores(g) | filler (proj/outproj) | AV(g-1)
"""
import numpy as np
import ml_dtypes

import concourse.bass as bass
import concourse.mybir as mybir
import concourse.tile as tile
from concourse import bacc
from concourse import dve_ops as _dve_ops
from concourse.bass_utils import run_bass_kernel_spmd
from concourse.dve_spec import Spec, Bin, AluOp, Src0, C0, One, lower, sq
from concourse.dve_uop import DveOpSpec

F32 = mybir.dt.float32
BF16 = mybir.dt.bfloat16
FP16 = mybir.dt.float16
NPBF16 = ml_dtypes.bfloat16

B, NQ_FULL, NK_FULL, D, H, DH = 4, 2048, 2048, 1024, 16, 64
SCALE = DH ** -0.5
MASK_NEG = -60.0
N_CORES = 8
HC = H // 2          # heads per core
HP = HC // 2         # head pairs per core
NQ = NQ_FULL         # q rows per core (all of the batch's queries)
QW = 512
QC = NQ // QW        # 4
IC = D // 128        # contraction chunks
KT_DVE = 2           # leading key tiles whose exp runs on the DVE


def _register_exp64():
    """Custom DVE op: exp(x*C0*64) ~= (1 + x*C0)^64, one fused uop chain.
    C0 = SCALE/64 so in0 can be the raw scores PSUM. Handles the -60 mask
    bias naturally (u stays positive, result underflows to ~0)."""
    name = "EXP_POW64_ANT"
    for o in _dve_ops.OPS:
        if o.name == name:
            return o
    body = Bin(AluOp.ADD, Bin(AluOp.MULTIPLY, Src0, C0), One)
    for _ in range(6):
        body = sq(body)

    def _ref(in0, in1, c0, c1, c2):
        u = in0.astype(np.float32) * np.float32(c0) + np.float32(1.0)
        for _ in range(6):
            u = u * u
        return u

    spec = Spec(body=body, reference=_ref)
    opcode = _dve_ops._CUSTOM_DVE_ROW_BASE + len(_dve_ops.OPS)
    shas = {}
    for ver in ("v3", "v4"):
        s = DveOpSpec(name=name, opcode=opcode, uops=lower(spec, ver=ver), rd1_en=False)
        shas[ver] = s.sha(ver)
    op = _dve_ops.DveOp(name, spec, subdim=False, uops_sha=shas)
    _dve_ops.OPS.append(op)
    _dve_ops._SUB_OPCODE_FOR_NAME[name] = opcode
    _dve_ops.CUSTOM_DVE_SPECS[name] = spec
    return op


EXP64 = _register_exp64()
C0_EXP = SCALE / 64.0


def _chunks(n, w=512):
    out, j = [], 0
    while j < n:
        out.append((j, min(w, n - j)))
        j += min(w, n - j)
    return out


def build_nc(nk, nbias=2, d=D):
    dh = DH
    NKT = nk // 128
    nbias = min(nbias, NKT)
    plain_end = NKT - nbias                      # kt < plain_end need no bias
    dve_upto = min(KT_DVE, plain_end) & ~1       # even # of leading DVE tiles

    nc = bacc.Bacc("TRN2", target_bir_lowering=False, debug=False)

    xq = nc.declare_dram_parameter("xq", [d, NQ], BF16, isOutput=False)
    xk = nc.declare_dram_parameter("xk", [d, nk], BF16, isOutput=False)
    xv = nc.declare_dram_parameter("xv", [d, nk], BF16, isOutput=False)
    wq = nc.declare_dram_parameter("wq", [d, 512], BF16, isOutput=False)
    wk = nc.declare_dram_parameter("wk", [d, 512], BF16, isOutput=False)
    wv = nc.declare_dram_parameter("wv", [d, 512], BF16, isOutput=False)
    wp = nc.declare_dram_parameter("wp", [512, d], BF16, isOutput=False)
    maskb = nc.declare_dram_parameter("maskb", [128, NKT], F32, isOutput=False)
    out = nc.declare_dram_parameter("out", [NQ, d], FP16, isOutput=True)

    with tile.TileContext(nc) as tc:
        with (
            tc.tile_pool(name="const", bufs=1) as cpool,
            tc.tile_pool(name="wpool", bufs=1) as wpool,
            tc.tile_pool(name="xpool", bufs=1) as xpool,
            tc.tile_pool(name="apool", bufs=1) as apool,
            tc.tile_pool(name="khp", bufs=2) as khpool,
            tc.tile_pool(name="qhp", bufs=2) as qhpool,
            tc.tile_pool(name="epool", bufs=4) as epool,
            tc.tile_pool(name="rpool", bufs=3) as rpool,
            tc.tile_pool(name="avn", bufs=2) as avnpool,
            tc.tile_pool(name="obuf", bufs=3) as obuf,
            tc.tile_pool(name="sc_ps", bufs=3, space="PSUM") as scps,
            tc.tile_pool(name="ps512", bufs=2, space="PSUM") as ps512,
        ):
            # ---- constants + input DMA in priority order ----
            maskb_s = cpool.tile([128, NKT], F32, tag="maskb")
            nc.sync.dma_start(out=maskb_s[:, :], in_=maskb[:, :])

            wk_s = wpool.tile([128, IC, 512], BF16, tag="wk")
            xkT = xpool.tile([128, IC, nk], BF16, tag="xk")
            for ic in range(IC):
                nc.sync.dma_start(out=wk_s[:, ic, :], in_=wk[ic * 128:(ic + 1) * 128, :])
                nc.sync.dma_start(out=xkT[:, ic, :], in_=xk[ic * 128:(ic + 1) * 128, :])
            wq_s = wpool.tile([128, IC, 512], BF16, tag="wq")
            xqT = xpool.tile([128, IC, NQ], BF16, tag="xq")
            for ic in range(IC):
                nc.sync.dma_start(out=wq_s[:, ic, :], in_=wq[ic * 128:(ic + 1) * 128, :])
                nc.sync.dma_start(
                    out=xqT[:, ic, 0:QW], in_=xq[ic * 128:(ic + 1) * 128, 0:QW]
                )
            for ic in range(IC):
                nc.sync.dma_start(
                    out=xqT[:, ic, QW:2 * QW], in_=xq[ic * 128:(ic + 1) * 128, QW:2 * QW]
                )
            wv_s = wpool.tile([128, IC, 512], BF16, tag="wv")
            xvT = xpool.tile([128, IC, nk], BF16, tag="xv")
            for ic in range(IC):
                nc.sync.dma_start(out=wv_s[:, ic, :], in_=wv[ic * 128:(ic + 1) * 128, :])
                nc.sync.dma_start(out=xvT[:, ic, :], in_=xv[ic * 128:(ic + 1) * 128, :])
            for qc in range(2, QC):
                for ic in range(IC):
                    nc.sync.dma_start(
                        out=xqT[:, ic, qc * QW:(qc + 1) * QW],
                        in_=xq[ic * 128:(ic + 1) * 128, qc * QW:(qc + 1) * QW],
                    )
            wp_s = wpool.tile([128, HP, d], BF16, tag="wp")
            for dc in range(HP):
                nc.sync.dma_start(out=wp_s[:, dc, :], in_=wp[dc * 128:(dc + 1) * 128, :])

            vh_s = apool.tile([128, NKT, HC, 128], BF16, tag="vh")
            nc.gpsimd.memset(vh_s[:, :, :, dh:], 1.0)
            avT_s = apool.tile([128, HP, NQ], BF16, tag="avT")

            khs = {}
            qhs = {}

            def emit_kproj(hp):
                kh_t = khpool.tile([128, nk], BF16, tag="kh", name=f"kh{hp}")
                for j0, jw in _chunks(nk):
                    ps = ps512.tile([128, 512], F32, tag="p", name=f"kps{hp}_{j0}")
                    for ic in range(IC):
                        nc.tensor.matmul(
                            ps[:, :jw],
                            wk_s[:, ic, hp * 128:(hp + 1) * 128],
                            xkT[:, ic, j0:j0 + jw],
                            start=(ic == 0), stop=(ic == IC - 1),
                        )
                    nc.scalar.activation(
                        kh_t[:, j0:j0 + jw], ps[:, :jw],
                        mybir.ActivationFunctionType.Copy, bias=0.0, scale=1.0,
                    )
                khs[hp] = kh_t

            def emit_qproj(hp, qc):
                if qc == 0:
                    qhs[hp] = qhpool.tile([128, NQ], BF16, tag="qh", name=f"qh{hp}")
                q0 = qc * QW
                ps = ps512.tile([128, 512], F32, tag="p", name=f"qps{hp}_{qc}")
                for ic in range(IC):
                    nc.tensor.matmul(
                        ps[:, :],
                        wq_s[:, ic, hp * 128:(hp + 1) * 128],
                        xqT[:, ic, q0:q0 + QW],
                        start=(ic == 0), stop=(ic == IC - 1),
                    )
                nc.scalar.activation(
                    qhs[hp][:, q0:q0 + QW], ps[:, :],
                    mybir.ActivationFunctionType.Copy, bias=0.0, scale=1.0,
                )

            def emit_vproj_kt(kt):
                ps = ps512.tile([128, 512], F32, tag="p", name=f"vps{kt}")
                for ic in range(IC):
                    nc.tensor.matmul(
                        ps[:, :],
                        xvT[:, ic, kt * 128:(kt + 1) * 128],
                        wv_s[:, ic, :],
                        start=(ic == 0), stop=(ic == IC - 1),
                    )
                nc.scalar.activation(
                    vh_s[:, kt, :, 0:dh],
                    ps[:, :].rearrange("p (a b) -> p a b", b=dh),
                    mybir.ActivationFunctionType.Copy, bias=0.0, scale=1.0,
                )

            kt_pairs = []
            j = 0
            while j < NKT:
                kt_pairs.append(tuple(range(j, min(j + 2, NKT))))
                j += 2

            def emit_scores(hp, qc):
                q0 = qc * QW
                kh_p, qh_p = khs[hp], qhs[hp]
                es = [
                    epool.tile([128, NKT, QW], BF16, tag="e", name=f"e{hp}_{qc}_{h}")
                    for h in range(2)
                ]
                for kts in kt_pairs:
                    pss = [
                        scps.tile([128, 2, 512], F32, tag="sc",
                                  name=f"sc{hp}_{qc}_{kts[0]}_{h}")
                        for h in range(2)
                    ]
                    for si, kt in enumerate(kts):
                        for half, ps in enumerate(pss):
                            p0 = half * 64
                            nc.tensor.matmul(
                                ps[:, si, :QW],
                                kh_p[p0:p0 + 64, kt * 128:(kt + 1) * 128],
                                qh_p[p0:p0 + 64, q0:q0 + QW],
                                start=True, stop=True,
                                tile_position=(p0, 0),
                            )
                    for half, e in enumerate(es):
                        ps = pss[half]
                        if len(kts) == 2 and kts[-1] < dve_upto:
                            nc.vector._custom_dve(
                                EXP64,
                                out=e[:, kts[0]:kts[0] + 2, :],
                                in0=ps[:, :, :QW],
                                s0=C0_EXP,
                            )
                        elif len(kts) == 2 and kts[-1] < plain_end:
                            nc.scalar.activation(
                                e[:, kts[0]:kts[0] + 2, :], ps[:, :, :QW],
                                mybir.ActivationFunctionType.Exp,
                                bias=0.0, scale=SCALE,
                            )
                        else:
                            for si, kt in enumerate(kts):
                                if kt >= plain_end:
                                    nc.scalar.activation(
                                        e[:, kt, :], ps[:, si, :QW],
                                        mybir.ActivationFunctionType.Exp,
                                        bias=maskb_s[:, kt:kt + 1], scale=SCALE,
                                    )
                                else:
                                    nc.scalar.activation(
                                        e[:, kt, :], ps[:, si, :QW],
                                        mybir.ActivationFunctionType.Exp,
                                        bias=0.0, scale=SCALE,
                                    )
                return es

            def emit_av(hp, qc, es):
                q0 = qc * QW
                for half, e in enumerate(es):
                    hh = 2 * hp + half
                    av = ps512.tile([128, 512], F32, tag="p", name=f"av{hp}_{qc}_{half}")
                    for kt in range(NKT):
                        nc.tensor.matmul(
                            av[:, :QW],
                            vh_s[:, kt, hh, :],
                            e[:, kt, :],
                            start=(kt == 0), stop=(kt == NKT - 1),
                        )
                    d64 = rpool.tile([128, QW], F32, tag="d64",
                                     name=f"d64_{hp}_{qc}_{half}")
                    nc.vector.tensor_copy(d64[64:128, :], av[64:128, :QW])
                    d0 = rpool.tile([64, QW], F32, tag="d0",
                                    name=f"d0_{hp}_{qc}_{half}")
                    nc.sync.dma_start(out=d0[:, :], in_=d64[64:128, :])
                    rb0 = rpool.tile([64, QW], F32, tag="rb0",
                                     name=f"rb0_{hp}_{qc}_{half}")
                    nc.vector.reciprocal_approx_fast(out=rb0[:, :], in_=d0[:, :])
                    if half == 0:
                        nc.vector.tensor_mul(
                            avT_s[0:dh, hp, q0:q0 + QW], av[0:dh, :QW], rb0[:, :]
                        )
                    else:
                        avn = avnpool.tile([dh, QW], BF16, tag="avn",
                                           name=f"avn{hp}_{qc}")
                        nc.vector.tensor_mul(avn[:, :], av[0:dh, :QW], rb0[:, :])
                        nc.sync.dma_start(
                            out=avT_s[64:128, hp, q0:q0 + QW], in_=avn[:, :]
                        )

            def emit_outproj(qc):
                for qt in range(4):
                    q0 = qc * QW + qt * 128
                    for o0, ow in _chunks(d):
                        ps = ps512.tile([128, 512], F32, tag="p",
                                        name=f"o{qc}_{qt}_{o0}")
                        for dc in range(HP):
                            nc.tensor.matmul(
                                ps[:, :ow],
                                avT_s[:, dc, q0:q0 + 128],
                                wp_s[:, dc, o0:o0 + ow],
                                start=(dc == 0), stop=(dc == HP - 1),
                            )
                        ot = obuf.tile([128, 512], FP16, tag="ot",
                                       name=f"ot{qc}_{qt}_{o0}")
                        nc.vector.tensor_copy(ot[:, :ow], ps[:, :ow])
                        nc.sync.dma_start(
                            out=out[q0:q0 + 128, o0:o0 + ow], in_=ot[:, :ow]
                        )

            def emit_filler(hp, qc):
                if hp == 0:
                    if qc == 0:
                        for kt in range(NKT):
                            emit_vproj_kt(kt)
                    elif qc == 1:
                        emit_qproj(0, 2)
                    elif qc == 2:
                        emit_qproj(0, 3)
                        emit_kproj(1)
                    else:
                        emit_qproj(1, 0)
                        emit_qproj(1, 1)
                elif hp < 3:
                    if qc <= 1:
                        emit_qproj(hp, qc + 2)
                    if qc == 1:
                        emit_kproj(hp + 1)
                    elif qc == 2:
                        emit_qproj(hp + 1, 0)
                    elif qc == 3:
                        emit_qproj(hp + 1, 1)
                else:
                    if qc <= 1:
                        emit_qproj(hp, qc + 2)

            # ---- main pipeline ----
            emit_kproj(0)
            emit_qproj(0, 0)
            emit_qproj(0, 1)
            prev = None
            out_pend = []
            for hp in range(HP):
                for qc in range(QC):
                    es = emit_scores(hp, qc)
                    emit_filler(hp, qc)
                    if out_pend:
                        emit_outproj(out_pend.pop(0))
                    if prev is not None:
                        emit_av(*prev)
                        if prev[0] == HP - 1:
                            out_pend.append(prev[1])
                    prev = (hp, qc, es)
            emit_av(*prev)
            out_pend.append(prev[1])
            for qc in out_pend:
                emit_outproj(qc)

    nc.compile()
    return nc


def host_prep(q, k, v, attention_mask, Wq, Wk, Wv, Wp, bp):
    """Pack valid keys, slice weights per head-half, build per-core inputs."""
    bsz, nk_full = attention_mask.shape
    idxs = [np.flatnonzero(attention_mask[b]) for b in range(bsz)]
    nv_min = min(len(ix) for ix in idxs)
    nk = max(128, -(-max(len(ix) for ix in idxs) // 128) * 128)
    nkt = nk // 128
    nbias = max(1, -(-(nk - nv_min) // 128))

    wqT = np.ascontiguousarray(Wq.T).astype(NPBF16)
    wkT = np.ascontiguousarray(Wk.T).astype(NPBF16)
    wvT = np.ascontiguousarray(Wv.T).astype(NPBF16)
    wpT = np.ascontiguousarray(Wp.T).astype(NPBF16)
    whalves = []
    for h in range(2):
        sl = slice(h * 512, (h + 1) * 512)
        whalves.append({
            "wq": np.ascontiguousarray(wqT[:, sl]),
            "wk": np.ascontiguousarray(wkT[:, sl]),
            "wv": np.ascontiguousarray(wvT[:, sl]),
            "wp": np.ascontiguousarray(wpT[sl, :]),
        })

    packed = []
    for b in range(bsz):
        ix = idxs[b]
        kp = np.zeros((nk, k.shape[2]), NPBF16)
        vp = np.zeros((nk, v.shape[2]), NPBF16)
        kp[:len(ix)] = k[b][ix].astype(NPBF16)
        vp[:len(ix)] = v[b][ix].astype(NPBF16)
        mb = np.full(nk, MASK_NEG, np.float32)
        mb[:len(ix)] = 0.0
        packed.append((
            np.ascontiguousarray(kp.T), np.ascontiguousarray(vp.T),
            np.ascontiguousarray(q[b].astype(NPBF16).T),
            np.ascontiguousarray(mb.reshape(nkt, 128).T),
        ))

    in_maps = []
    for c in range(N_CORES):
        b, half = divmod(c, 2)
        kpT, vpT, qT, mb = packed[b]
        in_maps.append({
            "xq": qT, "xk": kpT, "xv": vpT,
            "maskb": mb, **whalves[half],
        })
    return in_maps, nk, nbias


def assemble(results, bp):
    out = np.empty((B, NQ_FULL, D), np.float32)
    bp32 = np.asarray(bp, np.float32)
    for b in range(B):
        out[b] = (results[2 * b]["out"].astype(np.float32)
                  + results[2 * b + 1]["out"].astype(np.float32) + bp32)
    return out


_NC_CACHE = {}


def get_nc(nk, nbias=2):
    key = (nk, nbias)
    if key not in _NC_CACHE:
        _NC_CACHE[key] = build_nc(nk, nbias)
    return _NC_CACHE[key]


def kernel(q, k, v, attention_mask, Wq, Wk, Wv, Wp, bp):
    in_maps, nk, nbias = host_prep(q, k, v, attention_mask, Wq, Wk, Wv, Wp, bp)
    nc = get_nc(nk, nbias)
    res = run_bass_kernel_spmd(nc, in_maps, core_ids=list(range(N_CORES)))
    return assemble(res.results, bp)


# revision 9
# speedup vs baseline: 1.0918x; 1.0512x over previous
"""Bass/Tile Trainium2 kernel for nn_CrossAttention (B=4, Nq=Nk=2048, D=1024, H=16).

Sharding v2: 8 cores; core c handles batch b=c//2 and head-half (c%2)*8..+8,
over ALL 2048 query rows. K/V/Q projections are computed once per (batch, head)
with no duplication. The output projection contracts only this core's 512 head
dims, producing a partial [2048, 1024] fp16 output; the host adds the two
partials per batch plus the bias.

Ragged keys: valid keys packed on host (~Nk/2), tail tiles get a -60 additive
bias before exp.

Per-core pipeline (bf16 matmul operands, fp32 PSUM):
  scores per (head-pair, q-chunk): K=64 matmuls packed two heads per PE pass
    via tile_position row groups (concurrent pairs, ~2x)
  exp: leading KT_DVE key tiles via a custom DVE op (1+x/64)^64, remaining
    tiles (incl. biased tail) on the scalar engine -> both engines in parallel
  AV accumulates 9 key tiles; denominator via ones-block in vh
  software-pipelined emission: scores(g) | filler (proj/outproj) | AV(g-1)
"""
import numpy as np
import ml_dtypes

import concourse.bass as bass
import concourse.mybir as mybir
import concourse.tile as tile
from concourse import bacc
from concourse import dve_ops as _dve_ops
from concourse.bass_utils import run_bass_kernel_spmd
from concourse.dve_spec import Spec, Bin, AluOp, Src0, C0, One, lower, sq
from concourse.dve_uop import DveOpSpec

F32 = mybir.dt.float32
BF16 = mybir.dt.bfloat16
FP16 = mybir.dt.float16
NPBF16 = ml_dtypes.bfloat16

B, NQ_FULL, NK_FULL, D, H, DH = 4, 2048, 2048, 1024, 16, 64
SCALE = DH ** -0.5
MASK_NEG = -60.0
N_CORES = 8
HC = H // 2          # heads per core
HP = HC // 2         # head pairs per core
NQ = NQ_FULL         # q rows per core (all of the batch's queries)
QW = 512
QC = NQ // QW        # 4
IC = D // 128        # contraction chunks
KT_DVE = 2           # leading key tiles whose exp runs on the DVE


def _register_exp64():
    """Custom DVE op: exp(x*C0*64) ~= (1 + x*C0)^64, one fused uop chain.
    C0 = SCALE/64 so in0 can be the raw scores PSUM. Handles the -60 mask
    bias naturally (u stays positive, result underflows to ~0)."""
    name = "EXP_POW64_ANT"
    for o in _dve_ops.OPS:
        if o.name == name:
            return o
    body = Bin(AluOp.ADD, Bin(AluOp.MULTIPLY, Src0, C0), One)
    for _ in range(6):
        body = sq(body)

    def _ref(in0, in1, c0, c1, c2):
        u = in0.astype(np.float32) * np.float32(c0) + np.float32(1.0)
        for _ in range(6):
            u = u * u
        return u

    spec = Spec(body=body, reference=_ref)
    opcode = _dve_ops._CUSTOM_DVE_ROW_BASE + len(_dve_ops.OPS)
    shas = {}
    for ver in ("v3", "v4"):
        s = DveOpSpec(name=name, opcode=opcode, uops=lower(spec, ver=ver), rd1_en=False)
        shas[ver] = s.sha(ver)
    op = _dve_ops.DveOp(name, spec, subdim=False, uops_sha=shas)
    _dve_ops.OPS.append(op)
    _dve_ops._SUB_OPCODE_FOR_NAME[name] = opcode
    _dve_ops.CUSTOM_DVE_SPECS[name] = spec
    return op


EXP64 = _register_exp64()
C0_EXP = SCALE / 64.0


def _chunks(n, w=512):
    out, j = [], 0
    while j < n:
        out.append((j, min(w, n - j)))
        j += min(w, n - j)
    return out


def build_nc(nk, nbias=2, d=D):
    dh = DH
    NKT = nk // 128
    nbias = min(nbias, NKT)
    plain_end = NKT - nbias                      # kt < plain_end need no bias
    dve_upto = min(KT_DVE, plain_end) & ~1       # even # of leading DVE tiles

    nc = bacc.Bacc("TRN2", target_bir_lowering=False, debug=False)

    xq = nc.declare_dram_parameter("xq", [d, NQ], BF16, isOutput=False)
    xk = nc.declare_dram_parameter("xk", [d, nk], BF16, isOutput=False)
    xv = nc.declare_dram_parameter("xv", [d, nk], BF16, isOutput=False)
    wq = nc.declare_dram_parameter("wq", [d, 512], BF16, isOutput=False)
    wk = nc.declare_dram_parameter("wk", [d, 512], BF16, isOutput=False)
    wv = nc.declare_dram_parameter("wv", [d, 512], BF16, isOutput=False)
    wp = nc.declare_dram_parameter("wp", [512, d], BF16, isOutput=False)
    maskb = nc.declare_dram_parameter("maskb", [128, NKT], F32, isOutput=False)
    out = nc.declare_dram_parameter("out", [NQ, d], FP16, isOutput=True)

    with tile.TileContext(nc) as tc:
        with (
            tc.tile_pool(name="const", bufs=1) as cpool,
            tc.tile_pool(name="wpool", bufs=1) as wpool,
            tc.tile_pool(name="xpool", bufs=1) as xpool,
            tc.tile_pool(name="apool", bufs=1) as apool,
            tc.tile_pool(name="khp", bufs=2) as khpool,
            tc.tile_pool(name="qhp", bufs=2) as qhpool,
            tc.tile_pool(name="epool", bufs=4) as epool,
            tc.tile_pool(name="rpool", bufs=3) as rpool,
            tc.tile_pool(name="avn", bufs=2) as avnpool,
            tc.tile_pool(name="obuf", bufs=3) as obuf,
            tc.tile_pool(name="sc_ps", bufs=2, space="PSUM") as scps,
            tc.tile_pool(name="av_ps", bufs=2, space="PSUM") as avps,
            tc.tile_pool(name="po_ps", bufs=2, space="PSUM") as ps512,
        ):
            # ---- constants + input DMA in priority order ----
            maskb_s = cpool.tile([128, NKT], F32, tag="maskb")
            nc.sync.dma_start(out=maskb_s[:, :], in_=maskb[:, :])

            wk_s = wpool.tile([128, IC, 512], BF16, tag="wk")
            xkT = xpool.tile([128, IC, nk], BF16, tag="xk")
            for ic in range(IC):
                nc.sync.dma_start(out=wk_s[:, ic, :], in_=wk[ic * 128:(ic + 1) * 128, :])
                nc.sync.dma_start(out=xkT[:, ic, :], in_=xk[ic * 128:(ic + 1) * 128, :])
            wq_s = wpool.tile([128, IC, 512], BF16, tag="wq")
            xqT = xpool.tile([128, IC, NQ], BF16, tag="xq")
            for ic in range(IC):
                nc.sync.dma_start(out=wq_s[:, ic, :], in_=wq[ic * 128:(ic + 1) * 128, :])
                nc.sync.dma_start(
                    out=xqT[:, ic, 0:QW], in_=xq[ic * 128:(ic + 1) * 128, 0:QW]
                )
            for ic in range(IC):
                nc.sync.dma_start(
                    out=xqT[:, ic, QW:2 * QW], in_=xq[ic * 128:(ic + 1) * 128, QW:2 * QW]
                )
            wv_s = wpool.tile([128, IC, 512], BF16, tag="wv")
            xvT = xpool.tile([128, IC, nk], BF16, tag="xv")
            for ic in range(IC):
                nc.sync.dma_start(out=wv_s[:, ic, :], in_=wv[ic * 128:(ic + 1) * 128, :])
                nc.sync.dma_start(out=xvT[:, ic, :], in_=xv[ic * 128:(ic + 1) * 128, :])
            for qc in range(2, QC):
                for ic in range(IC):
                    nc.sync.dma_start(
                        out=xqT[:, ic, qc * QW:(qc + 1) * QW],
                        in_=xq[ic * 128:(ic + 1) * 128, qc * QW:(qc + 1) * QW],
                    )
            wp_s = wpool.tile([128, HP, d], BF16, tag="wp")
            for dc in range(HP):
                nc.sync.dma_start(out=wp_s[:, dc, :], in_=wp[dc * 128:(dc + 1) * 128, :])

            vh_s = apool.tile([128, NKT, HC, 128], BF16, tag="vh")
            nc.gpsimd.memset(vh_s[:, :, :, dh:], 1.0)
            avT_s = apool.tile([128, HP, NQ], BF16, tag="avT")

            khs = {}
            qhs = {}

            def emit_kproj(hp):
                kh_t = khpool.tile([128, nk], BF16, tag="kh", name=f"kh{hp}")
                for j0, jw in _chunks(nk):
                    ps = ps512.tile([128, 512], F32, tag="p", name=f"kps{hp}_{j0}")
                    for ic in range(IC):
                        nc.tensor.matmul(
                            ps[:, :jw],
                            wk_s[:, ic, hp * 128:(hp + 1) * 128],
                            xkT[:, ic, j0:j0 + jw],
                            start=(ic == 0), stop=(ic == IC - 1),
                        )
                    nc.scalar.activation(
                        kh_t[:, j0:j0 + jw], ps[:, :jw],
                        mybir.ActivationFunctionType.Copy, bias=0.0, scale=1.0,
                    )
                khs[hp] = kh_t

            def emit_qproj(hp, qc):
                if qc == 0:
                    qhs[hp] = qhpool.tile([128, NQ], BF16, tag="qh", name=f"qh{hp}")
                q0 = qc * QW
                ps = ps512.tile([128, 512], F32, tag="p", name=f"qps{hp}_{qc}")
                for ic in range(IC):
                    nc.tensor.matmul(
                        ps[:, :],
                        wq_s[:, ic, hp * 128:(hp + 1) * 128],
                        xqT[:, ic, q0:q0 + QW],
                        start=(ic == 0), stop=(ic == IC - 1),
                    )
                nc.scalar.activation(
                    qhs[hp][:, q0:q0 + QW], ps[:, :],
                    mybir.ActivationFunctionType.Copy, bias=0.0, scale=1.0,
                )

            def emit_vproj_kt(kt):
                ps = ps512.tile([128, 512], F32, tag="p", name=f"vps{kt}")
                for ic in range(IC):
                    nc.tensor.matmul(
                        ps[:, :],
                        xvT[:, ic, kt * 128:(kt + 1) * 128],
                        wv_s[:, ic, :],
                        start=(ic == 0), stop=(ic == IC - 1),
                    )
                nc.scalar.activation(
                    vh_s[:, kt, :, 0:dh],
                    ps[:, :].rearrange("p (a b) -> p a b", b=dh),
                    mybir.ActivationFunctionType.Copy, bias=0.0, scale=1.0,
                )

            kt_pairs = []
            j = 0
            while j < NKT:
                kt_pairs.append(tuple(range(j, min(j + 2, NKT))))
                j += 2

            def score_units(hp, qc):
                """5 thunks; each emits one kt-pair's 4 score MMs + the exps."""
                q0 = qc * QW
                es = [
                    epool.tile([128, NKT, QW], BF16, tag="e", name=f"e{hp}_{qc}_{h}")
                    for h in range(2)
                ]

                def mk(kts):
                    def thunk():
                        kh_p, qh_p = khs[hp], qhs[hp]
                        pss = [
                            scps.tile([128, 2, 512], F32, tag="sc",
                                      name=f"sc{hp}_{qc}_{kts[0]}_{h}")
                            for h in range(2)
                        ]
                        for si, kt in enumerate(kts):
                            for half, ps in enumerate(pss):
                                p0 = half * 64
                                nc.tensor.matmul(
                                    ps[:, si, :QW],
                                    kh_p[p0:p0 + 64, kt * 128:(kt + 1) * 128],
                                    qh_p[p0:p0 + 64, q0:q0 + QW],
                                    start=True, stop=True,
                                    tile_position=(p0, 0),
                                )
                        for half, e in enumerate(es):
                            ps = pss[half]
                            if len(kts) == 2 and kts[-1] < dve_upto:
                                nc.vector._custom_dve(
                                    EXP64,
                                    out=e[:, kts[0]:kts[0] + 2, :],
                                    in0=ps[:, :, :QW],
                                    s0=C0_EXP,
                                )
                            elif len(kts) == 2 and kts[-1] < plain_end:
                                nc.scalar.activation(
                                    e[:, kts[0]:kts[0] + 2, :], ps[:, :, :QW],
                                    mybir.ActivationFunctionType.Exp,
                                    bias=0.0, scale=SCALE,
                                )
                            else:
                                for si, kt in enumerate(kts):
                                    bias = (maskb_s[:, kt:kt + 1]
                                            if kt >= plain_end else 0.0)
                                    nc.scalar.activation(
                                        e[:, kt, :], ps[:, si, :QW],
                                        mybir.ActivationFunctionType.Exp,
                                        bias=bias, scale=SCALE,
                                    )
                    return thunk

                return [mk(kts) for kts in kt_pairs], es

            def av_units(hp, qc, es):
                """NKT thunks of 2 MMs (both heads, one kt) + epilogue thunk."""
                q0 = qc * QW
                avs = [
                    avps.tile([128, 512], F32, tag="av", name=f"av{hp}_{qc}_{h}")
                    for h in range(2)
                ]

                def mk(kt):
                    def thunk():
                        for half in range(2):
                            hh = 2 * hp + half
                            nc.tensor.matmul(
                                avs[half][:, :QW],
                                vh_s[:, kt, hh, :],
                                es[half][:, kt, :],
                                start=(kt == 0), stop=(kt == NKT - 1),
                            )
                        if kt == NKT - 1:
                            for half in range(2):
                                epilogue(half)
                    return thunk

                def epilogue(half):
                    av = avs[half]
                    d64 = rpool.tile([128, QW], F32, tag="d64",
                                     name=f"d64_{hp}_{qc}_{half}")
                    nc.vector.tensor_copy(d64[64:128, :], av[64:128, :QW])
                    d0 = rpool.tile([64, QW], F32, tag="d0",
                                    name=f"d0_{hp}_{qc}_{half}")
                    nc.sync.dma_start(out=d0[:, :], in_=d64[64:128, :])
                    rb0 = rpool.tile([64, QW], F32, tag="rb0",
                                     name=f"rb0_{hp}_{qc}_{half}")
                    nc.vector.reciprocal_approx_fast(out=rb0[:, :], in_=d0[:, :])
                    if half == 0:
                        nc.vector.tensor_mul(
                            avT_s[0:dh, hp, q0:q0 + QW], av[0:dh, :QW], rb0[:, :]
                        )
                    else:
                        avn = avnpool.tile([dh, QW], BF16, tag="avn",
                                           name=f"avn{hp}_{qc}")
                        nc.vector.tensor_mul(avn[:, :], av[0:dh, :QW], rb0[:, :])
                        nc.sync.dma_start(
                            out=avT_s[64:128, hp, q0:q0 + QW], in_=avn[:, :]
                        )

                return [mk(kt) for kt in range(NKT)]

            def outproj_units(qc):
                """8 thunks; each = one 4-MM chain + fp16 copy + DMA out."""
                def mk(qt, o0, ow):
                    def thunk():
                        q0 = qc * QW + qt * 128
                        ps = ps512.tile([128, 512], F32, tag="p",
                                        name=f"o{qc}_{qt}_{o0}")
                        for dc in range(HP):
                            nc.tensor.matmul(
                                ps[:, :ow],
                                avT_s[:, dc, q0:q0 + 128],
                                wp_s[:, dc, o0:o0 + ow],
                                start=(dc == 0), stop=(dc == HP - 1),
                            )
                        ot = obuf.tile([128, 512], FP16, tag="ot",
                                       name=f"ot{qc}_{qt}_{o0}")
                        nc.vector.tensor_copy(ot[:, :ow], ps[:, :ow])
                        nc.sync.dma_start(
                            out=out[q0:q0 + 128, o0:o0 + ow], in_=ot[:, :ow]
                        )
                    return thunk

                return [mk(qt, o0, ow) for qt in range(4) for o0, ow in _chunks(d)]

            def filler_units(hp, qc):
                th = []
                if hp == 0:
                    if qc == 0:
                        th += [(lambda kt=kt: emit_vproj_kt(kt)) for kt in range(NKT)]
                    elif qc == 1:
                        th.append(lambda: emit_qproj(0, 2))
                    elif qc == 2:
                        th.append(lambda: emit_qproj(0, 3))
                        th.append(lambda: emit_kproj(1))
                    else:
                        th.append(lambda: emit_qproj(1, 0))
                        th.append(lambda: emit_qproj(1, 1))
                elif hp < HP - 1:
                    if qc <= 1:
                        th.append(lambda hp=hp, qc=qc: emit_qproj(hp, qc + 2))
                    if qc == 1:
                        th.append(lambda hp=hp: emit_kproj(hp + 1))
                    elif qc == 2:
                        th.append(lambda hp=hp: emit_qproj(hp + 1, 0))
                    elif qc == 3:
                        th.append(lambda hp=hp: emit_qproj(hp + 1, 1))
                else:
                    if qc <= 1:
                        th.append(lambda hp=hp, qc=qc: emit_qproj(hp, qc + 2))
                return th

            def run_slot(sc_th, fill_th, av_th):
                """Interleave: filler first (early PSUM drain), then score units
                spaced by AV/filler work so scps recycling never stalls PE."""
                stream = []
                if fill_th:
                    stream.append(fill_th[0])
                rest = list(av_th) + list(fill_th[1:])
                ns = len(sc_th)
                stream += [sc_th[0], sc_th[1]] if ns >= 2 else sc_th[:1]
                k = 2
                # distribute `rest` between remaining score units
                per = max(1, len(rest) // max(1, ns - 1))
                ri = 0
                while k < ns:
                    stream += rest[ri:ri + per]
                    ri += per
                    stream.append(sc_th[k])
                    k += 1
                stream += rest[ri:]
                for t in stream:
                    t()

            # ---- main pipeline ----
            emit_kproj(0)
            emit_qproj(0, 0)
            emit_qproj(0, 1)
            prev = None
            out_pend = []
            for hp in range(HP):
                for qc in range(QC):
                    sc_th, es = score_units(hp, qc)
                    fill_th = filler_units(hp, qc)
                    if out_pend:
                        fill_th = fill_th + outproj_units(out_pend.pop(0))
                    av_th = av_units(*prev) if prev is not None else []
                    if prev is not None and prev[0] == HP - 1:
                        out_pend.append(prev[1])
                    run_slot(sc_th, fill_th, av_th)
                    prev = (hp, qc, es)
            for t in av_units(*prev):
                t()
            out_pend.append(prev[1])
            for qc in out_pend:
                for t in outproj_units(qc):
                    t()

    nc.compile()
    return nc


def host_prep(q, k, v, attention_mask, Wq, Wk, Wv, Wp, bp):
    """Pack valid keys, slice weights per head-half, build per-core inputs."""
    bsz, nk_full = attention_mask.shape
    idxs = [np.flatnonzero(attention_mask[b]) for b in range(bsz)]
    nv_min = min(len(ix) for ix in idxs)
    nk = max(128, -(-max(len(ix) for ix in idxs) // 128) * 128)
    nkt = nk // 128
    nbias = max(1, -(-(nk - nv_min) // 128))

    wqT = np.ascontiguousarray(Wq.T).astype(NPBF16)
    wkT = np.ascontiguousarray(Wk.T).astype(NPBF16)
    wvT = np.ascontiguousarray(Wv.T).astype(NPBF16)
    wpT = np.ascontiguousarray(Wp.T).astype(NPBF16)
    whalves = []
    for h in range(2):
        sl = slice(h * 512, (h + 1) * 512)
        whalves.append({
            "wq": np.ascontiguousarray(wqT[:, sl]),
            "wk": np.ascontiguousarray(wkT[:, sl]),
            "wv": np.ascontiguousarray(wvT[:, sl]),
            "wp": np.ascontiguousarray(wpT[sl, :]),
        })

    packed = []
    for b in range(bsz):
        ix = idxs[b]
        kp = np.zeros((nk, k.shape[2]), NPBF16)
        vp = np.zeros((nk, v.shape[2]), NPBF16)
        kp[:len(ix)] = k[b][ix].astype(NPBF16)
        vp[:len(ix)] = v[b][ix].astype(NPBF16)
        mb = np.full(nk, MASK_NEG, np.float32)
        mb[:len(ix)] = 0.0
        packed.append((
            np.ascontiguousarray(kp.T), np.ascontiguousarray(vp.T),
            np.ascontiguousarray(q[b].astype(NPBF16).T),
            np.ascontiguousarray(mb.reshape(nkt, 128).T),
        ))

    in_maps = []
    for c in range(N_CORES):
        b, half = divmod(c, 2)
        kpT, vpT, qT, mb = packed[b]
        in_maps.append({
            "xq": qT, "xk": kpT, "xv": vpT,
            "maskb": mb, **whalves[half],
        })
    return in_maps, nk, nbias


def assemble(results, bp):
    out = np.empty((B, NQ_FULL, D), np.float32)
    bp32 = np.asarray(bp, np.float32)
    for b in range(B):
        out[b] = (results[2 * b]["out"].astype(np.float32)
                  + results[2 * b + 1]["out"].astype(np.float32) + bp32)
    return out


_NC_CACHE = {}


def get_nc(nk, nbias=2):
    key = (nk, nbias)
    if key not in _NC_CACHE:
        _NC_CACHE[key] = build_nc(nk, nbias)
    return _NC_CACHE[key]


def kernel(q, k, v, attention_mask, Wq, Wk, Wv, Wp, bp):
    in_maps, nk, nbias = host_prep(q, k, v, attention_mask, Wq, Wk, Wv, Wp, bp)
    nc = get_nc(nk, nbias)
    res = run_bass_kernel_spmd(nc, in_maps, core_ids=list(range(N_CORES)))
    return assemble(res.results, bp)
